# revision 1
# baseline (speedup 1.0000x reference)
"""CFN cell on 8 TRN2 NeuronCores — tensor-parallel over H, fp32r matmuls.

Same compute structure as v3 (acts stationary, [Wtu|Weu] / [Wtw|Wew]
fused 512-wide moving operands, 1536 fp32r MMs/core), but every large
DRAM tensor is pre-packed on the host so each SBUF partition's data is
contiguous in DRAM:

    acts:    [n_win, 128p, kt, 256]   -> 8-16 KB DMA descriptors
    weights: [128p, kt, h2]           -> 8-32 KB descriptors
    sbh:     [n_win, 128p, 2, h_loc]

This lifts per-queue DMA throughput ~4x (descriptor-rate limited at 1 KB
lines) which removes the startup stall and mid-kernel PE starvation.
"""

import numpy as np
from contextlib import ExitStack

import concourse.bass as bass
import concourse.mybir as mybir
import concourse.tile as tile
from concourse import bacc
from concourse.bass_utils import run_bass_kernel_spmd

F32 = mybir.dt.float32
F32R = mybir.dt.float32r
AF = mybir.ActivationFunctionType
ALU = mybir.AluOpType

B, D_IN, H, NCORES = 4096, 2048, 2048, 8
H_LOC = H // NCORES  # 256
WIN = 256

TRACE = False
LAST_RESULTS = None
_NC_CACHE = {}


def build(nc, b, d_in, d_state, h_loc, act_bufs=3, pipe=5):
    n_bt = b // 128
    ktx, kts = d_in // 128, d_state // 128
    h2 = 2 * h_loc
    tpw = WIN // 128
    n_win = b // WIN

    xt = nc.dram_tensor("xt", [n_win, 128, ktx, WIN], F32R,
                        kind="ExternalInput").ap()
    st = nc.dram_tensor("st", [n_win, 128, kts, WIN], F32R,
                        kind="ExternalInput").ap()
    sbh = nc.dram_tensor("sbh", [n_win, 128, tpw, h_loc], F32,
                         kind="ExternalInput").ap()
    wte = nc.dram_tensor("wte", [128, kts, h2], F32R, kind="ExternalInput").ap()
    wtwe = nc.dram_tensor("wtwe", [128, ktx, h2], F32R, kind="ExternalInput").ap()
    wwx = nc.dram_tensor("wwx", [128, ktx, h_loc], F32R, kind="ExternalInput").ap()
    bias = nc.dram_tensor("bias", [h2], F32, kind="ExternalInput").ap()
    out = nc.dram_tensor("h_out", [b, h_loc], F32, kind="ExternalOutput").ap()

    with tile.TileContext(nc) as tc, ExitStack() as ctx:
        consts = ctx.enter_context(tc.tile_pool(name="consts", bufs=1))
        acts = ctx.enter_context(tc.tile_pool(name="acts", bufs=act_bufs))
        temps = ctx.enter_context(tc.tile_pool(name="temps", bufs=2))
        psum = ctx.enter_context(tc.tile_pool(name="psum", bufs=1, space="PSUM"))

        stw_map, xtw_map, sbh_map = {}, {}, {}

        def load_stw(iw, nq=2, eng=None):
            eng = eng or nc.sync
            stw = acts.tile([128, kts, WIN], F32R, tag="stw", name=f"stw{iw}")
            step = max(kts // nq, 1)
            for c in range(0, kts, step):
                ce = min(c + step, kts)
                eng.dma_start(out=stw[:, c:ce, :], in_=st[iw, :, c:ce, :])
            stw_map[iw] = stw

        def load_xtw(iw, nq=2, eng=None):
            eng = eng or nc.sync
            xtw = acts.tile([128, ktx, WIN], F32R, tag="xtw", name=f"xtw{iw}")
            step = max(ktx // nq, 1)
            for c in range(0, ktx, step):
                ce = min(c + step, ktx)
                eng.dma_start(out=xtw[:, c:ce, :], in_=xt[iw, :, c:ce, :])
            xtw_map[iw] = xtw

        def load_sbh(iw):
            sbh_t = acts.tile([128, tpw, h_loc], F32, tag="sbh_t", name=f"sbh{iw}")
            nc.gpsimd.dma_start(out=sbh_t, in_=sbh[iw])
            sbh_map[iw] = sbh_t

        # Startup choreography: feed the PE in consumption order.  s-phases
        # need stw + wte only, so those stream first (spread wide across
        # queues); the x-phase operands follow while the prologue s-phases
        # keep the PE busy.
        wte_sb = consts.tile([128, kts, h2], F32R, tag="wte")
        wtwe_sb = consts.tile([128, ktx, h2], F32R, tag="wtwe")
        wwx_sb = consts.tile([128, ktx, h_loc], F32R, tag="wwx")

        # s-phase consumption order: stw0, then wte chunks interleaved with
        # the stw windows the prologue s-phases will need next
        load_stw(0, nq=4)
        n_pre_win = min(act_bufs, n_win)
        pre_stw = []
        for iw in range(1, n_pre_win):
            stw = acts.tile([128, kts, WIN], F32R, tag="stw", name=f"stw{iw}")
            stw_map[iw] = stw
            pre_stw.append((iw, stw))
        for c in range(0, kts, 2):
            ce = min(c + 2, kts)
            nc.sync.dma_start(out=wte_sb[:, c:ce, :], in_=wte[:, c:ce, :])
            for iw, stw in pre_stw:
                nc.sync.dma_start(out=stw[:, c:ce, :], in_=st[iw, :, c:ce, :])
        bias_bc = consts.tile([128, h2], F32, tag="bias_bc")
        bias_bcast_ap = bass.AP(
            tensor=bias.tensor, offset=bias.offset,
            ap=[[0, 128]] + list(bias.ap),
        )
        nc.gpsimd.dma_start(out=bias_bc, in_=bias_bcast_ap)
        load_sbh(0)
        load_xtw(0, nq=4)
        for c in range(0, ktx, 2):
            ce = min(c + 2, ktx)
            nc.sync.dma_start(out=wtwe_sb[:, c:ce, :], in_=wtwe[:, c:ce, :])
            nc.sync.dma_start(out=wwx_sb[:, c:ce, :], in_=wwx[:, c:ce, :])
        for iw in range(1, n_pre_win):
            load_xtw(iw)
            load_sbh(iw)

        def get_window(iw):
            if iw not in stw_map:
                load_stw(iw)
                load_xtw(iw)
                load_sbh(iw)
            for m in (stw_map, xtw_map, sbh_map):
                for key in [k for k in m if k < iw - act_bufs]:
                    del m[key]
            return stw_map[iw], xtw_map[iw], sbh_map[iw]

        def s_phase(ib):
            stw, _, _ = get_window(ib // tpw)
            bcol = (ib % tpw) * 128
            ps_te = psum.tile([128, h2], F32, tag="ps_te", bufs=pipe + 1,
                              name=f"ps_te{ib}")
            for k in range(kts):
                nc.tensor.matmul(
                    ps_te, stw[:, k, bcol:bcol + 128], wte_sb[:, k, :],
                    start=(k == 0), stop=False,
                )
            return ps_te

        def x_phase_and_epilogue(ib, ps_te):
            bsl = slice(ib * 128, (ib + 1) * 128)
            _, xtw, sbh_t = get_window(ib // tpw)
            it = ib % tpw
            bcol = it * 128
            ps_w = psum.tile([128, h_loc], F32, tag="ps_w", bufs=2,
                             name=f"ps_w{ib}")
            for k in range(ktx):
                nc.tensor.matmul(
                    ps_te, xtw[:, k, bcol:bcol + 128], wtwe_sb[:, k, :],
                    start=False, stop=(k == ktx - 1),
                )
                nc.tensor.matmul(
                    ps_w, xtw[:, k, bcol:bcol + 128], wwx_sb[:, k, :],
                    start=(k == 0), stop=(k == ktx - 1),
                )

            tsh = temps.tile([128, h_loc], F32, tag="tsh", name=f"tsh{ib}")
            nc.scalar.activation(tsh, sbh_t[:, it, :], AF.Tanh)
            pre = temps.tile([128, h2], F32, tag="pre", name=f"pre{ib}")
            nc.vector.scalar_tensor_tensor(
                pre, ps_te, 1.0, bias_bc, op0=ALU.mult, op1=ALU.add,
            )
            theta = temps.tile([128, h_loc], F32, tag="theta", name=f"th{ib}")
            nc.scalar.activation(theta, pre[:, :h_loc], AF.Sigmoid)
            eta = temps.tile([128, h_loc], F32, tag="eta", name=f"et{ib}")
            nc.scalar.activation(eta, pre[:, h_loc:], AF.Sigmoid)
            twx = temps.tile([128, h_loc], F32, tag="twx", name=f"twx{ib}")
            nc.scalar.activation(twx, ps_w, AF.Tanh)

            p1 = temps.tile([128, h_loc], F32, tag="p1", name=f"p1{ib}")
            nc.vector.tensor_mul(p1, theta, tsh)
            p2 = temps.tile([128, h_loc], F32, tag="p2", name=f"p2{ib}")
            nc.vector.tensor_mul(p2, eta, twx)
            ho = temps.tile([128, h_loc], F32, tag="ho", name=f"ho{ib}")
            nc.vector.tensor_add(ho, p1, p2)
            nc.gpsimd.dma_start(out=out[bsl, :], in_=ho)

        pending = [(ib, s_phase(ib)) for ib in range(min(pipe, n_bt))]
        for ib in range(pipe, n_bt):
            pib, ps = pending.pop(0)
            x_phase_and_epilogue(pib, ps)
            pending.append((ib, s_phase(ib)))
        for pib, ps in pending:
            x_phase_and_epilogue(pib, ps)

    nc.compile()
    return nc


def _get_nc():
    key = (B, D_IN, H, H_LOC)
    if key not in _NC_CACHE:
        nc = bacc.Bacc("TRN2", target_bir_lowering=False, debug=False,
                       num_devices=NCORES)
        _NC_CACHE[key] = build(nc, B, D_IN, H, H_LOC)
    return _NC_CACHE[key]


def _pack_acts(at):  # at: [D, B] (transposed activations)
    d, b_ = at.shape
    kt, n_win = d // 128, b_ // WIN
    # (t*128+p, iw*WIN+j) -> [iw, p, t, j]
    return np.ascontiguousarray(
        at.reshape(kt, 128, n_win, WIN).transpose(2, 1, 0, 3)
    )


def _pack_w(wm):  # wm: [D, h] -> [p, t, h]
    d, h = wm.shape
    kt = d // 128
    return np.ascontiguousarray(wm.reshape(kt, 128, h).transpose(1, 0, 2))


def make_in_maps(inputs):
    x = np.ascontiguousarray(np.asarray(inputs["inputs"], dtype=np.float32))
    s = np.ascontiguousarray(np.asarray(inputs["state"], dtype=np.float32))
    w = {
        k: np.asarray(inputs[k], dtype=np.float32)
        for k in ("theta_u_w", "theta_w_w", "eta_u_w", "eta_w_w", "wx_w")
    }
    bt_full = np.asarray(inputs["theta_w_b"], dtype=np.float32)
    be_full = np.asarray(inputs["eta_w_b"], dtype=np.float32)

    xt_p = _pack_acts(x.T)  # shared by all cores
    st_p = _pack_acts(s.T)
    n_win, tpw = B // WIN, WIN // 128

    in_maps = []
    for c in range(NCORES):
        hsl = slice(c * H_LOC, (c + 1) * H_LOC)
        sbh_c = np.ascontiguousarray(
            s[:, hsl].reshape(n_win, tpw, 128, H_LOC).transpose(0, 2, 1, 3)
        )
        in_maps.append({
            "xt": xt_p,
            "st": st_p,
            "sbh": sbh_c,
            "wte": _pack_w(np.concatenate(
                [w["theta_u_w"][:, hsl], w["eta_u_w"][:, hsl]], axis=1)),
            "wtwe": _pack_w(np.concatenate(
                [w["theta_w_w"][:, hsl], w["eta_w_w"][:, hsl]], axis=1)),
            "wwx": _pack_w(w["wx_w"][:, hsl]),
            "bias": np.ascontiguousarray(
                np.concatenate([bt_full[hsl], be_full[hsl]])
            ),
        })
    return in_maps


def kernel(**inputs):
    global LAST_RESULTS
    in_maps = make_in_maps(inputs)
    nc = _get_nc()
    res = run_bass_kernel_spmd(nc, in_maps, core_ids=list(range(NCORES)),
                               trace=TRACE)
    LAST_RESULTS = res

    h = np.empty((B, H), np.float32)
    for c in range(NCORES):
        h[:, c * H_LOC:(c + 1) * H_LOC] = res.results[c]["h_out"]
    return (h, h)



# revision 2
# speedup vs baseline: 1.1348x; 1.1348x over previous
"""CFN cell on 8 TRN2 NeuronCores — tensor-parallel over H, bf16 matmuls.

v2: same compute structure as v1 (acts stationary, [Wtu|Weu] / [Wtw|Wew]
fused 512-wide moving operands), but all matmul operands are bf16:

  - bf16 LDWEIGHTS is ~107 ns (vs ~190 ns for fp32r), so the 256-wide
    wwx matmuls (107 ns) no longer expose weight-load time — this was
    ~35-40 us of PE overhead in the fp32r version.
  - activation/weight DMA halves (64 MB -> 32 MB per core), shrinking
    the startup stall.

Accuracy: bf16 quantization of x/s/weights gives ~3e-3 rel error on h
(tolerance 2e-2). PSUM accumulation stays fp32; epilogue math fp32.
"""

import numpy as np
import ml_dtypes
from contextlib import ExitStack

import concourse.bass as bass
import concourse.mybir as mybir
import concourse.tile as tile
from concourse import bacc
from concourse.bass_utils import run_bass_kernel_spmd

F32 = mybir.dt.float32
BF16 = mybir.dt.bfloat16
AF = mybir.ActivationFunctionType
ALU = mybir.AluOpType

B, D_IN, H, NCORES = 4096, 2048, 2048, 8
H_LOC = H // NCORES  # 256
WIN = 256

TRACE = False
LAST_RESULTS = None
_NC_CACHE = {}


def build(nc, b, d_in, d_state, h_loc, act_bufs=3, pipe=5):
    n_bt = b // 128
    ktx, kts = d_in // 128, d_state // 128
    h2 = 2 * h_loc
    tpw = WIN // 128
    n_win = b // WIN

    xt = nc.dram_tensor("xt", [n_win, 128, ktx, WIN], BF16,
                        kind="ExternalInput").ap()
    st = nc.dram_tensor("st", [n_win, 128, kts, WIN], BF16,
                        kind="ExternalInput").ap()
    sbh = nc.dram_tensor("sbh", [n_win, 128, tpw, h_loc], BF16,
                         kind="ExternalInput").ap()
    wte = nc.dram_tensor("wte", [128, kts, h2], BF16, kind="ExternalInput").ap()
    wtwe = nc.dram_tensor("wtwe", [128, ktx, h2], BF16,
                          kind="ExternalInput").ap()
    wwx = nc.dram_tensor("wwx", [128, ktx, h_loc], BF16,
                         kind="ExternalInput").ap()
    bias = nc.dram_tensor("bias", [h2], F32, kind="ExternalInput").ap()
    out = nc.dram_tensor("h_out", [b, h_loc], F32, kind="ExternalOutput").ap()

    with tile.TileContext(nc) as tc, ExitStack() as ctx:
        consts = ctx.enter_context(tc.tile_pool(name="consts", bufs=1))
        acts = ctx.enter_context(tc.tile_pool(name="acts", bufs=act_bufs))
        temps = ctx.enter_context(tc.tile_pool(name="temps", bufs=2))
        psum = ctx.enter_context(tc.tile_pool(name="psum", bufs=1, space="PSUM"))

        stw_map, xtw_map, sbh_map = {}, {}, {}

        def load_stw(iw, nq=2, eng=None):
            eng = eng or nc.sync
            stw = acts.tile([128, kts, WIN], BF16, tag="stw", name=f"stw{iw}")
            step = max(kts // nq, 1)
            for c in range(0, kts, step):
                ce = min(c + step, kts)
                eng.dma_start(out=stw[:, c:ce, :], in_=st[iw, :, c:ce, :])
            stw_map[iw] = stw

        def load_xtw(iw, nq=2, eng=None):
            eng = eng or nc.sync
            xtw = acts.tile([128, ktx, WIN], BF16, tag="xtw", name=f"xtw{iw}")
            step = max(ktx // nq, 1)
            for c in range(0, ktx, step):
                ce = min(c + step, ktx)
                eng.dma_start(out=xtw[:, c:ce, :], in_=xt[iw, :, c:ce, :])
            xtw_map[iw] = xtw

        def load_sbh(iw):
            sbh_t = acts.tile([128, tpw, h_loc], BF16, tag="sbh_t",
                              name=f"sbh{iw}")
            nc.gpsimd.dma_start(out=sbh_t, in_=sbh[iw])
            sbh_map[iw] = sbh_t

        # Startup choreography: feed the PE in consumption order.  s-phases
        # need stw + wte only, so those stream first (spread wide across
        # queues); the x-phase operands follow while the prologue s-phases
        # keep the PE busy.
        wte_sb = consts.tile([128, kts, h2], BF16, tag="wte")
        wtwe_sb = consts.tile([128, ktx, h2], BF16, tag="wtwe")
        wwx_sb = consts.tile([128, ktx, h_loc], BF16, tag="wwx")

        # s-phase consumption order: stw0, then wte chunks interleaved with
        # the stw windows the prologue s-phases will need next
        load_stw(0, nq=4)
        n_pre_win = min(act_bufs, n_win)
        pre_stw = []
        for iw in range(1, n_pre_win):
            stw = acts.tile([128, kts, WIN], BF16, tag="stw", name=f"stw{iw}")
            stw_map[iw] = stw
            pre_stw.append((iw, stw))
        for c in range(0, kts, 2):
            ce = min(c + 2, kts)
            nc.sync.dma_start(out=wte_sb[:, c:ce, :], in_=wte[:, c:ce, :])
            for iw, stw in pre_stw:
                nc.sync.dma_start(out=stw[:, c:ce, :], in_=st[iw, :, c:ce, :])
        bias_bc = consts.tile([128, h2], F32, tag="bias_bc")
        bias_bcast_ap = bass.AP(
            tensor=bias.tensor, offset=bias.offset,
            ap=[[0, 128]] + list(bias.ap),
        )
        nc.gpsimd.dma_start(out=bias_bc, in_=bias_bcast_ap)
        load_sbh(0)
        load_xtw(0, nq=4)
        for c in range(0, ktx, 2):
            ce = min(c + 2, ktx)
            nc.sync.dma_start(out=wtwe_sb[:, c:ce, :], in_=wtwe[:, c:ce, :])
            nc.sync.dma_start(out=wwx_sb[:, c:ce, :], in_=wwx[:, c:ce, :])
        for iw in range(1, n_pre_win):
            load_xtw(iw)
            load_sbh(iw)

        def get_window(iw):
            if iw not in stw_map:
                load_stw(iw)
                load_xtw(iw)
                load_sbh(iw)
            for m in (stw_map, xtw_map, sbh_map):
                for key in [k for k in m if k < iw - act_bufs]:
                    del m[key]
            return stw_map[iw], xtw_map[iw], sbh_map[iw]

        def s_phase(ib):
            stw, _, _ = get_window(ib // tpw)
            bcol = (ib % tpw) * 128
            ps_te = psum.tile([128, h2], F32, tag="ps_te", bufs=pipe + 1,
                              name=f"ps_te{ib}")
            for k in range(kts):
                nc.tensor.matmul(
                    ps_te, stw[:, k, bcol:bcol + 128], wte_sb[:, k, :],
                    start=(k == 0), stop=False,
                )
            return ps_te

        def x_phase_and_epilogue(ib, ps_te):
            bsl = slice(ib * 128, (ib + 1) * 128)
            _, xtw, sbh_t = get_window(ib // tpw)
            it = ib % tpw
            bcol = it * 128
            ps_w = psum.tile([128, h_loc], F32, tag="ps_w", bufs=2,
                             name=f"ps_w{ib}")
            for k in range(ktx):
                nc.tensor.matmul(
                    ps_te, xtw[:, k, bcol:bcol + 128], wtwe_sb[:, k, :],
                    start=False, stop=(k == ktx - 1),
                )
                nc.tensor.matmul(
                    ps_w, xtw[:, k, bcol:bcol + 128], wwx_sb[:, k, :],
                    start=(k == 0), stop=(k == ktx - 1),
                )

            tsh = temps.tile([128, h_loc], F32, tag="tsh", name=f"tsh{ib}")
            nc.scalar.activation(tsh, sbh_t[:, it, :], AF.Tanh)
            pre = temps.tile([128, h2], F32, tag="pre", name=f"pre{ib}")
            nc.vector.scalar_tensor_tensor(
                pre, ps_te, 1.0, bias_bc, op0=ALU.mult, op1=ALU.add,
            )
            theta = temps.tile([128, h_loc], F32, tag="theta", name=f"th{ib}")
            nc.scalar.activation(theta, pre[:, :h_loc], AF.Sigmoid)
            eta = temps.tile([128, h_loc], F32, tag="eta", name=f"et{ib}")
            nc.scalar.activation(eta, pre[:, h_loc:], AF.Sigmoid)
            twx = temps.tile([128, h_loc], F32, tag="twx", name=f"twx{ib}")
            nc.scalar.activation(twx, ps_w, AF.Tanh)

            p1 = temps.tile([128, h_loc], F32, tag="p1", name=f"p1{ib}")
            nc.vector.tensor_mul(p1, theta, tsh)
            p2 = temps.tile([128, h_loc], F32, tag="p2", name=f"p2{ib}")
            nc.vector.tensor_mul(p2, eta, twx)
            ho = temps.tile([128, h_loc], F32, tag="ho", name=f"ho{ib}")
            nc.vector.tensor_add(ho, p1, p2)
            nc.gpsimd.dma_start(out=out[bsl, :], in_=ho)

        pending = [(ib, s_phase(ib)) for ib in range(min(pipe, n_bt))]
        for ib in range(pipe, n_bt):
            pib, ps = pending.pop(0)
            x_phase_and_epilogue(pib, ps)
            pending.append((ib, s_phase(ib)))
        for pib, ps in pending:
            x_phase_and_epilogue(pib, ps)

    nc.compile()
    return nc


def _get_nc():
    key = (B, D_IN, H, H_LOC)
    if key not in _NC_CACHE:
        nc = bacc.Bacc("TRN2", target_bir_lowering=False, debug=False,
                       num_devices=NCORES)
        _NC_CACHE[key] = build(nc, B, D_IN, H, H_LOC)
    return _NC_CACHE[key]


BF = ml_dtypes.bfloat16


def _pack_acts(at):  # at: [D, B] (transposed activations), bf16
    d, b_ = at.shape
    kt, n_win = d // 128, b_ // WIN
    # (t*128+p, iw*WIN+j) -> [iw, p, t, j]
    return np.ascontiguousarray(
        at.reshape(kt, 128, n_win, WIN).transpose(2, 1, 0, 3)
    )


def _pack_w(wm):  # wm: [D, h] -> [p, t, h], bf16
    d, h = wm.shape
    kt = d // 128
    return np.ascontiguousarray(wm.reshape(kt, 128, h).transpose(1, 0, 2))


def make_in_maps(inputs):
    x = np.asarray(inputs["inputs"], dtype=np.float32)
    s = np.asarray(inputs["state"], dtype=np.float32)
    xb = x.astype(BF)
    sb = s.astype(BF)
    w = {
        k: np.asarray(inputs[k], dtype=np.float32).astype(BF)
        for k in ("theta_u_w", "theta_w_w", "eta_u_w", "eta_w_w", "wx_w")
    }
    bt_full = np.asarray(inputs["theta_w_b"], dtype=np.float32)
    be_full = np.asarray(inputs["eta_w_b"], dtype=np.float32)

    xt_p = _pack_acts(np.ascontiguousarray(xb.T))  # shared by all cores
    st_p = _pack_acts(np.ascontiguousarray(sb.T))
    n_win, tpw = B // WIN, WIN // 128

    in_maps = []
    for c in range(NCORES):
        hsl = slice(c * H_LOC, (c + 1) * H_LOC)
        sbh_c = np.ascontiguousarray(
            sb[:, hsl].reshape(n_win, tpw, 128, H_LOC).transpose(0, 2, 1, 3)
        )
        in_maps.append({
            "xt": xt_p,
            "st": st_p,
            "sbh": sbh_c,
            "wte": _pack_w(np.concatenate(
                [w["theta_u_w"][:, hsl], w["eta_u_w"][:, hsl]], axis=1)),
            "wtwe": _pack_w(np.concatenate(
                [w["theta_w_w"][:, hsl], w["eta_w_w"][:, hsl]], axis=1)),
            "wwx": _pack_w(w["wx_w"][:, hsl]),
            "bias": np.ascontiguousarray(
                np.concatenate([bt_full[hsl], be_full[hsl]])
            ),
        })
    return in_maps


def kernel(**inputs):
    global LAST_RESULTS
    in_maps = make_in_maps(inputs)
    nc = _get_nc()
    res = run_bass_kernel_spmd(nc, in_maps, core_ids=list(range(NCORES)),
                               trace=TRACE)
    LAST_RESULTS = res

    h = np.empty((B, H), np.float32)
    for c in range(NCORES):
        h[:, c * H_LOC:(c + 1) * H_LOC] = res.results[c]["h_out"]
    return (h, h)
